# revision 6
# baseline (speedup 1.0000x reference)
"""v7: fp16 q+v projection, fp8-DoubleRow k projection, fp16 attention.

Single-head causal attention (B=8, T=2048, E=1024, H=64) on 8 trn2 cores,
data-parallel over batch. Precision layout chosen from host error sims
(gate rel_err < 2e-2, measured 0.0112 for this config):
  - q and v projected in fp16 (one shared stationary [Wq|Wv], full psum),
  - k projected in fp8e4m3 DoubleRow (pair-packed contraction, 2x fewer
    PE cycles), evacuated to fp8,
  - scoresT computed in fp8 DoubleRow over h-pairs (q,k repacked to
    [32, 2, T] via SBUF->SBUF DMA, pair mapping h = p + 32k),
  - exp on ACT -> wT fp16; PV (w @ [v|1]) in fp16; f32 normalize.

Device schedule per core:
  proj (overlapped with pipelined x DMAs; qv j-outer 0..7 / k g-outer 0..3
  into 8 psum banks):
    pqv[c] += wqv_j^T @ x16_j[:, chunk]      (fp16)
    pk[c]  += wk8_g^T @ x8_g[:, :, chunk]    (fp8 DoubleRow)
    evacuate: q->fp8, k->fp8, v->fp16 (rows 64:127); repack q8p/k8p via DMA;
    v1[s, 65] = PE-transpose(v) + ones column (rowsum trick)
  attention (j = key block 0..15, PV lagging one block):
    scoresT = k8p_j^T @ q8p (DoubleRow) -> exp((1/8)x) on ACT -> wT_j fp16
    diag tri-mask; outT[65, t] += v1_j^T @ wT_j (psum bank per 512-chunk)
    as chunk c completes: PE-transpose outT, reciprocal-normalize, store f32
"""

import numpy as np
import ml_dtypes

import concourse.bass as bass
import concourse.mybir as mybir
from concourse.tile import TileContext
from concourse.bass_utils import run_bass_kernel_spmd

B, T, E, H = 8, 2048, 1024, 64
NT = T // 128   # 16 key/row blocks
NE = E // 128   # 8 fp16 contraction blocks
NG = E // 256   # 4 fp8 pair-contraction groups
NC = T // 512   # 4 column chunks
F16 = mybir.dt.float16
F32 = mybir.dt.float32
F8 = mybir.dt.float8e4
NP8 = ml_dtypes.float8_e4m3
SCALE = float(H) ** -0.5
DR = mybir.MatmulPerfMode.DoubleRow


def _split_excess_waits(nc: bass.Bass, cap: int = 1) -> int:
    n_split = 0
    for f in nc.m.functions:
        for bb in f.blocks:
            insts = list(bb.instructions)
            out = []
            dirty = False
            for inst in insts:
                si = inst.sync_info
                waits = list(si.on_wait) if si and si.on_wait else []
                if len(waits) > cap:
                    si.on_wait = waits[:cap]
                    for w in waits[cap:]:
                        nop = mybir.InstNoOp(
                            name=f"I-waitsplit-{n_split}", ins=[], outs=[]
                        )
                        nop.engine = inst.engine
                        nop.sync_info = mybir.SyncInfo(on_wait=[w], on_update=[])
                        out.append(nop)
                        n_split += 1
                    dirty = True
                out.append(inst)
            if dirty:
                bb.instructions = out
    return n_split


def build_nc(split_waits: bool = True) -> bass.Bass:
    nc = bass.Bass()
    x16 = nc.dram_tensor("x16", [E, T], F16, kind="ExternalInput")
    x8 = nc.dram_tensor("x8", [128, NG * 2 * T], F8, kind="ExternalInput")
    wqv16 = nc.dram_tensor("wqv16", [128, NE * 128], F16, kind="ExternalInput")
    wk8 = nc.dram_tensor("wk8", [128, NG * 2 * H], F8, kind="ExternalInput")
    eye64v_d = nc.dram_tensor("eye64v", [128, 64], F16, kind="ExternalInput")
    eye32_d = nc.dram_tensor("eye32", [128, 128], F32, kind="ExternalInput")
    tri_d = nc.dram_tensor("tri", [128, 128], F16, kind="ExternalInput")
    out = nc.dram_tensor("out", [T, H], F32, kind="ExternalOutput")
    x16_ap, x8_ap, out_ap = x16.ap(), x8.ap(), out.ap()

    with TileContext(nc) as tc:
        with (
            tc.tile_pool(name="const", bufs=1) as cpool,
            tc.tile_pool(name="wts", bufs=1) as wpool,
            tc.tile_pool(name="xt", bufs=8) as xtpool,
            tc.tile_pool(name="x8t", bufs=4) as x8pool,
            tc.tile_pool(name="qkv", bufs=1) as qkvpool,
            tc.tile_pool(name="wTp", bufs=4) as wtpool,
            tc.tile_pool(name="fin", bufs=2) as finpool,
        ):
            wqv_t = wpool.tile([128, NE * 128], F16, tag="wqv")
            nc.sync.dma_start(wqv_t[:], wqv16.ap())
            wk_t = wpool.tile([128, NG * 2 * H], F8, tag="wk")
            nc.sync.dma_start(wk_t[:], wk8.ap())

            xts = [
                xtpool.tile([128, T], F16, tag="xt", name=f"xt{j}")
                for j in range(NE)
            ]
            x8ts = [
                x8pool.tile([128, 2 * T], F8, tag="x8t", name=f"x8t{g}")
                for g in range(NG)
            ]
            x8_r = x8_ap.rearrange("p (g k t) -> p g k t", g=NG, k=2)

            def load_x16(j):
                for h in range(2):
                    sl = slice(1024 * h, 1024 * h + 1024)
                    nc.sync.dma_start(
                        xts[j][:, sl], x16_ap[128 * j : 128 * j + 128, sl]
                    )

            def load_x8(g):
                xg = x8ts[g][:].rearrange("p (k t) -> p k t", k=2)
                for h in range(2):
                    sl = slice(1024 * h, 1024 * h + 1024)
                    nc.sync.dma_start(xg[:, :, sl], x8_r[:, g, :, sl])

            load_x16(0)
            load_x16(1)

            eye64v = cpool.tile([128, 64], F16, tag="eye64v")
            nc.sync.dma_start(eye64v[:], eye64v_d.ap())
            eye32 = cpool.tile([128, 128], F32, tag="eye32")
            nc.sync.dma_start(eye32[:], eye32_d.ap())
            tri = cpool.tile([128, 128], F16, tag="tri")
            nc.sync.dma_start(tri[:], tri_d.ap())
            zb = cpool.tile([128, 1], F32, tag="zb")
            nc.gpsimd.memset(zb[:], 0.0)
            expwarm = cpool.tile([128, 1], F16, tag="expwarm")
            nc.scalar.activation(
                expwarm[:], zb[:], mybir.ActivationFunctionType.Exp,
                bias=zb[:, 0:1], scale=1.0,
            )

            q16 = qkvpool.tile([64, T], F16, tag="q16")
            k16 = qkvpool.tile([64, T], F16, tag="k16")
            vTT = qkvpool.tile([128, T], F16, tag="vTT")
            v1 = qkvpool.tile([128, NT * 65], F16, tag="v1")
            nc.gpsimd.memset(
                v1[:].rearrange("p (i c) -> p i c", c=65)[:, :, 64:65], 1.0
            )

            wk_r = wk_t[:].rearrange("p (g k m) -> p g k m", g=NG, k=2)

            # ---------- projections ----------
            with tc.tile_pool(name="ps12", bufs=1, space="PSUM") as ps12:
                pp = [
                    ps12.tile([128, 512], F32, tag="pp", bufs=8, name=f"pp{i}")
                    for i in range(8)
                ]
                # pp[0..3]: qv chunks (fp16); pp[4..7]: k chunks (fp8 DR)
                for j in range(NE):
                    if j + 2 < NE:
                        load_x16(j + 2)
                    if j % 2 == 0:
                        load_x8(j // 2)
                    for half in range(2):
                        for c in (2 * half, 2 * half + 1):
                            nc.tensor.matmul(
                                pp[c][:],
                                wqv_t[:, 128 * j : 128 * j + 128],
                                xts[j][:, 512 * c : 512 * c + 512],
                                start=(j == 0), stop=(j == NE - 1),
                            )
                    if j % 2 == 1:
                        g = j // 2
                        xg = x8ts[g][:].rearrange("p (k t) -> p k t", k=2)
                        for half in range(2):
                            for c in (2 * half, 2 * half + 1):
                                nc.tensor.matmul(
                                    pp[4 + c][0:64, :],
                                    wk_r[:, g],
                                    xg[:, :, 512 * c : 512 * c + 512],
                                    start=(g == 0), stop=(g == NG - 1),
                                    perf_mode=DR,
                                )

                # evacuate: q,k -> fp16 at partitions 0:63, v -> fp16 rows 64:127
                for c in range(NC):
                    sl = slice(512 * c, 512 * c + 512)
                    if c % 2 == 0:
                        nc.vector.tensor_copy(q16[:, sl], pp[c][0:64, :])
                        nc.scalar.copy(k16[:, sl], pp[4 + c][0:64, :])
                        nc.scalar.copy(vTT[64:128, sl], pp[c][64:128, :])
                    else:
                        nc.scalar.copy(q16[:, sl], pp[c][0:64, :])
                        nc.vector.tensor_copy(k16[:, sl], pp[4 + c][0:64, :])
                        nc.vector.tensor_copy(vTT[64:128, sl], pp[c][64:128, :])

            # ---------- attention: scores, exp, PV, finish ----------
            with tc.tile_pool(name="ps3", bufs=1, space="PSUM") as ps3:
                otps = [
                    ps3.tile([65, 512], F32, tag="ot", bufs=4, name=f"ot{c}")
                    for c in range(NC)
                ]
                wTs = {}

                def emit_pv(j):
                    s0 = 128 * j
                    for c in range(s0 // 512, NC):
                        t0 = max(512 * c, s0)
                        t1 = 512 * c + 512
                        nc.tensor.matmul(
                            otps[c][:, t0 - 512 * c : 512],
                            v1[:, 65 * j : 65 * j + 65],
                            wTs[j][:, t0 - s0 : t1 - s0],
                            start=(j == 0), stop=(j == 4 * c + 3),
                        )

                def emit_finish(c):
                    oc = finpool.tile(
                        [65, 512], F32, tag="oc", bufs=2, name=f"oc{c}"
                    )
                    nc.vector.tensor_copy(oc[:, 0:256], otps[c][:, 0:256])
                    nc.scalar.copy(oc[:, 256:512], otps[c][:, 256:512])
                    ft = ps3.tile(
                        [128, 260], F32, tag="ot", bufs=4, name=f"ft{c}"
                    )
                    for i in range(4):
                        nc.tensor.transpose(
                            ft[:, 65 * i : 65 * i + 65],
                            oc[:, 128 * i : 128 * i + 128],
                            eye32[0:65, 0:65],
                        )
                    rcp = finpool.tile(
                        [128, 4], F32, tag="rcp", bufs=2, name=f"rcp{c}"
                    )
                    nc.vector.reciprocal(
                        rcp[:],
                        ft[:].rearrange("p (i c) -> p i c", c=65)[:, :, 64:65],
                    )
                    ob = finpool.tile(
                        [128, 256], F32, tag="ob", bufs=2, name=f"ob{c}"
                    )
                    for i in range(4):
                        if i % 2 == 0:
                            nc.vector.tensor_scalar_mul(
                                ob[:, 64 * i : 64 * i + 64],
                                ft[:, 65 * i : 65 * i + 64],
                                rcp[:, i : i + 1],
                            )
                        else:
                            nc.scalar.mul(
                                ob[:, 64 * i : 64 * i + 64],
                                ft[:, 65 * i : 65 * i + 64],
                                rcp[:, i : i + 1],
                            )
                    nc.sync.dma_start(
                        out_ap[512 * c : 512 * c + 512, :].rearrange(
                            "(i p) h -> p i h", p=128
                        ),
                        ob[:].rearrange("p (i h) -> p i h", h=64),
                    )

                for j in range(NT):
                    s0 = 128 * j
                    span = T - s0
                    wT = wtpool.tile(
                        [128, T], F16, tag="wT", bufs=4, name=f"wT{j}"
                    )
                    wTs[j] = wT
                    off = 0
                    while off < span:
                        w = min(1024, span - off)
                        sc = ps3.tile(
                            [128, 1024], F32, tag="sc", bufs=2,
                            name=f"sc{j}_{off}",
                        )
                        o2 = 0
                        while o2 < w:
                            n = min(512, w - o2)
                            t0 = s0 + off + o2
                            nc.tensor.matmul(
                                sc[:, o2 : o2 + n],
                                k16[:, s0 : s0 + 128],
                                q16[:, t0 : t0 + n],
                                start=True, stop=True,
                            )
                            o2 += n
                        nc.scalar.activation(
                            wT[:, off : off + w], sc[:, 0:w],
                            mybir.ActivationFunctionType.Exp,
                            bias=zb[:, 0:1], scale=SCALE,
                        )
                        if off == 0:
                            nc.vector.tensor_mul(
                                wT[:, 0:128], wT[:, 0:128], tri[:]
                            )
                        off += w
                    if j <= 1:
                        g = j
                        tp = ps3.tile(
                            [128, 1024], F32, tag="sc", bufs=2, name=f"tp{g}"
                        )
                        for i in range(8):
                            blk = 8 * g + i
                            nc.tensor.matmul(
                                tp[:, 64 * i : 64 * i + 64],
                                vTT[64:128, 128 * blk : 128 * blk + 128],
                                eye64v[64:128, :],
                                start=True, stop=True,
                            )
                        nc.vector.tensor_copy(
                            v1[:, 520 * g : 520 * g + 520].rearrange(
                                "p (i c) -> p i c", c=65
                            )[:, :, 0:64],
                            tp[:, 0:512].rearrange("p (i c) -> p i c", c=64),
                        )
                    if j >= 1:
                        emit_pv(j - 1)
                        if (j - 1) % 4 == 3:
                            emit_finish((j - 1) // 4)
                emit_pv(NT - 1)
                emit_finish(NC - 1)

    if split_waits:
        _split_excess_waits(nc)
    return nc


_NC_CACHE = None


def _get_nc() -> bass.Bass:
    global _NC_CACHE
    if _NC_CACHE is None:
        _NC_CACHE = build_nc()
    return _NC_CACHE


def _pack_pairs(arr, m):
    """[E, m] -> [128, NG*2*m] with E = 256g + 2p + k on (partition p, pair k)."""
    return np.ascontiguousarray(
        arr.reshape(NG, 128, 2, m).transpose(1, 0, 2, 3).reshape(128, NG * 2 * m)
    )


def kernel(x, Wq, Wk, Wv, **run_kwargs):
    nc = _get_nc()
    x = np.asarray(x)
    wqv_full = np.concatenate(
        [np.asarray(Wq), np.asarray(Wv)], axis=1
    ).astype(np.float16)
    wqv_sw = np.ascontiguousarray(
        wqv_full.reshape(NE, 128, 128).transpose(1, 0, 2).reshape(128, NE * 128)
    )
    wk_sw = _pack_pairs(np.asarray(Wk).astype(NP8), H)
    eye64v = np.concatenate(
        [np.zeros((64, 64), np.float16), np.eye(64, dtype=np.float16)], axis=0
    )
    eye32 = np.eye(128, dtype=np.float32)
    tri = np.triu(np.ones((128, 128), dtype=np.float16))
    in_maps = []
    for b in range(B):
        xtb = np.ascontiguousarray(x[b].T)
        in_maps.append({
            "x16": xtb.astype(np.float16),
            "x8": _pack_pairs(xtb.astype(NP8), T),
            "wqv16": wqv_sw,
            "wk8": wk_sw,
            "eye64v": eye64v,
            "eye32": eye32,
            "tri": tri,
        })
    res = run_bass_kernel_spmd(nc, in_maps, core_ids=list(range(B)), **run_kwargs)
    out = np.stack([res.results[b]["out"] for b in range(B)], axis=0)
    kernel.last_results = res
    return out
